# revision 30
# baseline (speedup 1.0000x reference)
"""EuclideanLossWithOHEM on 8 trn2 NeuronCores (Bass, hand-synchronized).

Sharding: pure data-parallel over batch N=16 -> 2 samples per core.

Math (per sample n, labels k in [0,9), 0 = background):
    s2(pix)   = (pred0-gt_df0)^2 + (pred1-gt_df1)^2
    c_k       = #pixels with label k (host bincount, exact)
    posCount  = sum_{k>=1} c_k,  segRemain = #{k>=1: c_k>0}
    segAve    = posCount/segRemain,  alpha_k = segAve/c_k, alpha_0 = 1
With this input distribution 3*posCount >> c_0, so OHEM keeps every
negative pixel and
    num  = sum_pix alpha_{x} * s2 = S_tot + sum_pix delta_{x} * s2
    den  = posCount + min(3*posCount, c_0)
    loss = sum_n num_n / N / 2 / (2 * sum_n den_n)
The centered per-pixel map delta = alpha-1 (|delta| ~ 1% for uniform
labels) is built on host and shipped as fp8-e4m3 scaled by 64. A
first-order host correction (exact counts x device S_tot/HW) cancels
the fp8 table rounding; the residual is ~1e-6 relative. A host
fallback reproduces exact reference semantics whenever the
keep-all-negatives assumption does not hold (checked exactly on host).

Device program (no TileContext; static per-chunk buffers, manual sems):
    host packs per chunk a contiguous [128 x (4w + w/2)] f16 block
    [p0|p1|g0|g1|64*delta(fp8)], so each chunk is ONE HWDGE DMA (sync
    engine), all 8 issued back-to-back at program start.
    DVE : d01 = p01-g01 (f16 2x);  s2 = e0+e1 (2x);
          stt: s2 * delta-fp8(bitcast), accum_out -> sum(delta*s2)
    ACT : e01 = Square(d01), accum_out -> sum(s2)  (table pre-warmed)
Every chunk has its own SBUF buffers -> no WAR hazards; only RAW sync:
    blk DMA -> sub (semb_i), sub -> square (semv), square -> fold+stt
    (sema), stt -> store (semt), store -> end (semst). Each semaphore is
    cleared by the engine that waits on it, as its first instruction.
NOTE: non-uniform chunk widths reproducibly wedge the NTFF profiler /
re-execution path on this stack -- keep WIDTHS uniform.
"""

import numpy as np

# ---- problem constants (hardcoded per contract) ----
N_FULL = 16
C = 2
H = 512
W = 512
HW = H * W
NCORES = 8
S = N_FULL // NCORES      # samples per core = 2
NL = 9                    # labels 0..8
NP_RATIO = 3

# ---- kernel layout knobs ----
FP = HW // 128            # pixels per partition per sample = 2048
WSC = 64.0                # delta-map scale (keeps fp8 values normal)
# variable chunk widths: small first (HBM warm-up), small last (short tail)
WIDTHS0 = [512, 512, 512, 512]
WIDTHS1 = [512, 512, 512, 512]
NCH = len(WIDTHS0)
NCHT = S * NCH            # total chunks per core
assert sum(WIDTHS0) == FP and sum(WIDTHS1) == FP


def _line(w):
    return 4 * w + w // 2  # f16 slots per chunk line: p0|p1|g0|g1|delta-fp8


# global chunk table per core: (sample, pixel offset, width, column offset)
CHUNKS = []
_off = 0
for _s, _ws in enumerate([WIDTHS0, WIDTHS1]):
    _fl = 0
    for _w in _ws:
        CHUNKS.append((_s, _fl, _w, _off))
        _fl += _w
        _off += _line(_w)
TOT = _off
SW = TOT // S             # f16 slots per sample

_cache = {}


def _build_nc():
    import concourse.bass as bass
    import concourse.mybir as mybir

    f32 = mybir.dt.float32
    f16 = mybir.dt.float16
    Alu = mybir.AluOpType
    Act = mybir.ActivationFunctionType

    fp8 = mybir.dt.float8e4

    nc = bass.Bass("TRN2", target_bir_lowering=False, debug=False)

    # host-packed flat stream: per chunk a CONTIGUOUS [128, line] block of
    # [p0 | p1 | g0 | g1 | 64*delta(fp8)] lines
    pg = nc.dram_tensor("pg", [128 * TOT], f16, kind="ExternalInput").ap()
    # acc cols: [NCHT x sum(delta*s2) | NCHT x sum(s2)]
    acc_d = nc.dram_tensor("acc", [128, 2 * NCHT], f32, kind="ExternalOutput").ap()

    blk = [nc.alloc_sbuf_tensor(f"blk{i}", [128, _line(w)], f16).ap()
           for i, (_s, _fl, w, _o) in enumerate(CHUNKS)]
    d01 = [nc.alloc_sbuf_tensor(f"d01_{i}", [128, 2 * w], f16).ap()
           for i, (_s, _fl, w, _o) in enumerate(CHUNKS)]
    e01 = [nc.alloc_sbuf_tensor(f"e01_{i}", [128, 2 * w], f16).ap()
           for i, (_s, _fl, w, _o) in enumerate(CHUNKS)]
    s2 = [nc.alloc_sbuf_tensor(f"s2_{i}", [128, w], f16).ap()
          for i, (_s, _fl, w, _o) in enumerate(CHUNKS)]
    junk = nc.alloc_sbuf_tensor("junk", [128, max(w for _s, _f, w, _o in CHUNKS)], f16).ap()
    acc = nc.alloc_sbuf_tensor("acc_sb", [128, 2 * NCHT], f32).ap()
    warm = nc.alloc_sbuf_tensor("warm", [128, 1], f16).ap()

    def colW(i):
        return i

    def colT(i):
        return NCHT + i

    # semaphore blocks, each cleared by the engine that WAITS on it, as its
    # first instruction -- no cross-engine init barrier needed (all bumps
    # happen several microseconds after every engine has run its clear).
    semb = [nc.alloc_semaphore(f"semb{i}") for i in range(NCHT)]  # DVE waits
    semg = nc.alloc_semaphore("semg")     # GpSimd add completions (DVE waits)
    sema = nc.alloc_semaphore("sema")     # ACT square completions (GpS waits)
    semv = nc.alloc_semaphore("semv")     # DVE sub completions (ACT waits)
    semt = nc.alloc_semaphore("semt")     # DVE stt completions (SP waits)
    semst = nc.alloc_semaphore("semst")   # store completions (SP waits)
    nums = [s.num for s in semb] + [semg.num, sema.num, semv.num,
                                    semt.num, semst.num]
    assert nums == list(range(nums[0], nums[0] + len(nums)))

    # ---- SP: issue every chunk load immediately; clear its own sems
    # afterwards (they are not waited on until the first store, ~15us in) ----
    for i, (_s, _fl, w, off) in enumerate(CHUNKS):
        ln = _line(w)
        src_ap = pg[128 * off:128 * (off + ln)].rearrange("(p f) -> p f", p=128)
        nc.sync.dma_start(blk[i], src_ap).then_inc(semb[i], 16)
    nc.sync.sem_clear(range(semt.num, semst.num + 1))

    # ---- DVE: software-pipelined sub / fold / stt (lookahead 2) ----
    nc.vector.sem_clear(range(semb[0].num, sema.num + 1))

    def emit_sub(i):
        w = CHUNKS[i][2]
        nc.vector.wait_ge(semb[i], 16)
        nc.vector.tensor_tensor(
            d01[i], blk[i][:, 0:2 * w], blk[i][:, 2 * w:4 * w], Alu.subtract
        ).then_inc(semv, 1)

    def emit_tail(i):
        w = CHUNKS[i][2]
        nc.vector.wait_ge(sema, i + 1)
        nc.vector.tensor_tensor(
            s2[i], e01[i][:, 0:w], e01[i][:, w:2 * w], Alu.add
        )
        nc.vector.scalar_tensor_tensor(
            junk[:, 0:w], s2[i], 1.0, blk[i][:, 4 * w:_line(w)].bitcast(fp8),
            op0=Alu.bypass, op1=Alu.mult,
            accum_out=acc[:, colW(i):colW(i) + 1],
        ).then_inc(semt, 1)

    LOOK = 1
    for i in range(NCHT):
        emit_sub(i)
        if i >= LOOK:
            emit_tail(i - LOOK)
    for i in range(NCHT - LOOK, NCHT):
        emit_tail(i)

    # ---- ACT: warm the Square table during the first load, then squares ----
    nc.scalar.sem_clear(range(semv.num, semv.num + 1))
    nc.scalar.activation(warm, junk[:, 0:1], Act.Square)
    for i in range(NCHT):
        nc.scalar.wait_ge(semv, i + 1)
        nc.scalar.activation(
            e01[i], d01[i], Act.Square,
            accum_out=acc[:, colT(i):colT(i) + 1],
        ).then_inc(sema, 1)

    # ---- SP: one store at the end, then wait for it to land ----
    nc.sync.wait_ge(semt, NCHT)
    nc.sync.dma_start(acc_d, acc).then_inc(semst, 16)
    nc.sync.wait_ge(semst, 16)

    return nc


def _reference_fallback(pred, gt_df, gt):
    """Exact numpy replica of the reference (used only if the OHEM
    keep-all-negatives assumption is violated)."""
    pred = np.asarray(pred, np.float32)
    gt_df = np.asarray(gt_df, np.float32)
    g = np.asarray(gt)[:, 0]
    N = pred.shape[0]
    distL2 = (pred - gt_df).astype(np.float32) ** 2
    counts = np.stack([np.bincount(x.ravel(), minlength=NL)[:NL] for x in g])
    pos_counts = counts.copy()
    pos_counts[:, 0] = 0
    posCount = pos_counts.sum(1).astype(np.float32)
    segRemain = (pos_counts > 0).sum(1).astype(np.float32)
    segAve = np.where(segRemain > 0, posCount / np.maximum(segRemain, 1.0), 0.0)
    cnt = np.take_along_axis(counts, g.reshape(N, -1), axis=1).reshape(g.shape)
    weight = np.where(
        g > 0, segAve[:, None, None] / np.maximum(cnt, 1.0), 0.0
    ).astype(np.float32)
    regionNeg = (weight == 0).astype(np.float32)
    sumPos = (weight > 0).sum((1, 2))
    sumNeg = regionNeg.sum((1, 2))
    sumhardNeg = np.minimum(NP_RATIO * sumPos, sumNeg).astype(np.int64)
    lossNeg = (distL2[:, 0] + distL2[:, 1]) * regionNeg
    flat = lossNeg.reshape(N, -1)
    order = np.argsort(flat, axis=1, kind="stable")
    ranks = np.empty_like(order)
    np.put_along_axis(ranks, order, np.arange(flat.shape[1])[None, :], axis=1)
    keep = ranks >= (flat.shape[1] - sumhardNeg)[:, None]
    lossHard = np.where(keep, flat, 0.0)
    weightNeg = (lossHard != 0).astype(np.float32).reshape(lossNeg.shape)
    wTot = weight + weightNeg
    num = float((distL2 * wTot[:, None]).sum(dtype=np.float64))
    den = 2.0 * float(wTot.sum(dtype=np.float64))
    return np.float32(num / N / 2.0 / den)


def kernel(pred, gt_df, gt):
    from concourse.bass_utils import run_bass_kernel_spmd

    pred = np.ascontiguousarray(np.asarray(pred, np.float32))
    gt_df = np.ascontiguousarray(np.asarray(gt_df, np.float32))
    g = np.asarray(gt).reshape(N_FULL, H, W)

    # ---- host label statistics (exact) ----
    if not (g.min() >= 0 and g.max() < NL):
        return _reference_fallback(pred, gt_df, gt)
    counts = np.stack(
        [np.bincount(x.ravel().astype(np.int64), minlength=NL)[:NL] for x in g]
    ).astype(np.float64)                                   # (N, NL)
    posCount = counts[:, 1:].sum(1)                        # (N,)
    segRemain = (counts[:, 1:] > 0).sum(1)
    sumhard = np.minimum(NP_RATIO * posCount, counts[:, 0])
    # keep-all-negatives assumption: OHEM keeps every background pixel
    if not np.all((sumhard == counts[:, 0]) & (posCount > 0)):
        return _reference_fallback(pred, gt_df, gt)

    import ml_dtypes

    segAve = posCount / np.maximum(segRemain, 1)
    # delta_k = alpha_k - 1 (centered); alpha_0 = 1 -> delta_0 = 0
    delta = np.zeros((N_FULL, NL), np.float64)
    nzmask = counts[:, 1:] > 0
    delta[:, 1:][nzmask] = (
        segAve[:, None] / np.where(nzmask, counts[:, 1:], 1.0)
    )[nzmask] - 1.0
    # scaled by WSC so fp8 values stay in the normal range
    delta8 = (delta * WSC).astype(np.float32).astype(ml_dtypes.float8_e4m3)
    delta8_f64 = delta8.astype(np.float64) / WSC           # what the HW sees

    # ---- pack [p0 | p1 | g0 | g1 | 64*delta(fp8)] per chunk line ----
    pred_r = pred.reshape(N_FULL, C, 128, FP)
    gtdf_r = gt_df.reshape(N_FULL, C, 128, FP)
    gr = g.reshape(N_FULL, 128, FP)
    # per sample: list of contiguous [128, line] blocks, flattened
    pgs = np.empty((N_FULL, SW * 128), np.float16)
    for n in range(N_FULL):
        widths = WIDTHS0 if n % 2 == 0 else WIDTHS1
        d8 = delta8[n][gr[n]]                              # (128, FP) fp8
        fl = 0
        off = 0
        for w in widths:
            ln = 4 * w + w // 2
            block = pgs[n, off * 128:(off + ln) * 128].reshape(128, ln)
            block[:, 0 * w:1 * w] = pred_r[n, 0, :, fl:fl + w]
            block[:, 1 * w:2 * w] = pred_r[n, 1, :, fl:fl + w]
            block[:, 2 * w:3 * w] = gtdf_r[n, 0, :, fl:fl + w]
            block[:, 3 * w:4 * w] = gtdf_r[n, 1, :, fl:fl + w]
            block[:, 4 * w:ln].view(np.uint16)[:] = (
                np.ascontiguousarray(d8[:, fl:fl + w]).view(np.uint16)
            )
            fl += w
            off += ln

    if "nc" not in _cache:
        _cache["nc"] = _build_nc()
    nc = _cache["nc"]

    in_maps = []
    for c in range(NCORES):
        lo, hi = c * S, (c + 1) * S
        in_maps.append({
            "pg": np.ascontiguousarray(
                np.concatenate([pgs[lo], pgs[lo + 1]])),
        })
    res = run_bass_kernel_spmd(nc, in_maps, core_ids=list(range(NCORES)))
    _cache["last_results"] = res
    _cache["last_in_maps"] = in_maps

    # ---- host-side combine (f64) ----
    num = 0.0
    den_w = 0.0
    for c in range(NCORES):
        out = res.results[c]
        aa = np.asarray(out["acc"], np.float64)            # [128, 2*NCHT]
        for s in range(S):
            n = c * S + s
            dotW = aa[:, s * NCH:(s + 1) * NCH].sum() / WSC
            S_tot = aa[:, NCHT + s * NCH:NCHT + (s + 1) * NCH].sum()
            # first-order correction for fp8 rounding of the delta table:
            # num_exact - (S_tot+dot) = sum_k (delta_k - fp8) * S_k, and
            # S_k ~= c_k * mean(s2) with mean(s2) = S_tot / HW.
            corr = float(
                ((delta[n] - delta8_f64[n]) * counts[n]).sum()
            ) * (S_tot / HW)
            num += S_tot + dotW + corr
            den_w += posCount[n] + sumhard[n]

    loss = num / N_FULL / 2.0 / (2.0 * den_w)
    return np.float32(loss)
